# revision 29
# baseline (speedup 1.0000x reference)
"""Streaming-block GNN pool kernel (best config): bf16 single-pass matmul,
ACT relu+bias evac to bf16, all-DVE half-fold tensor_tensor trees (2x_1P
bf16 = 4 reads/cycle) reducing each 16-group to 2 partials; the host folds
the last pair and converts sum -> mean. GPSIMD stays idle: its SBUF port is
shared with the DVE and concurrent use inflates both by 30-70%."""
import sys

sys.path.insert(0, "/opt/trn_rl_repo")

import numpy as np
import ml_dtypes

import concourse.bass as bass
import concourse.bacc as bacc
import concourse.tile as tile
from concourse import mybir
from concourse.bass_utils import run_bass_kernel_spmd

N_CORES = 8
IN_DIM = 128
OUT_DIM = 512
N_OBS = 25000
M_LANES = 400000
GS = 16
M_C = M_LANES // N_CORES
G_C = N_OBS // N_CORES
N_CHUNK = OUT_DIM // 128
BLK = 2048
TW = 2   # max tail width left for the host fold
SW = 4   # sum tail width (one fewer DVE level; host folds 4)

MODE = "treev9"
DVE_EVAC_MOD = 48  # every Nth chunk-span evac'd on DVE instead of ACT

_compiled = {}


def _build(mode: str) -> bass.Bass:
    nc = bacc.Bacc(None, target_bir_lowering=False)
    f32 = mybir.dt.float32
    bf16 = mybir.dt.bfloat16
    AL = mybir.AluOpType

    xth_d = nc.dram_tensor("xth", [IN_DIM, M_C], bf16, kind="ExternalInput")
    wth_d = nc.dram_tensor("wth", [IN_DIM, OUT_DIM], bf16, kind="ExternalInput")
    bsc_d = nc.dram_tensor("bsc", [128, N_CHUNK], f32, kind="ExternalInput")
    omax_d = nc.dram_tensor("omax", [OUT_DIM, G_C, TW], bf16, kind="ExternalOutput")
    osum_d = nc.dram_tensor("osum", [OUT_DIM, G_C, SW], bf16, kind="ExternalOutput")

    with nc.allow_low_precision("bf16 pooled outputs; host upconverts"), \
            tile.TileContext(nc) as tc:
        with (
            tc.tile_pool(name="singles", bufs=1) as singles,
            tc.tile_pool(name="xin", bufs=2) as xin,
            tc.tile_pool(name="rsb", bufs=2) as rsb,
            tc.tile_pool(name="acc", bufs=1) as accp,
            tc.tile_pool(name="dtmp", bufs=1) as dtmp,
            tc.tile_pool(name="psum", bufs=2, space="PSUM") as psum,
        ):
            starts = [0, 512]
            while starts[-1] + BLK < M_C:
                starts.append(starts[-1] + BLK)
            blocks = [(s, min(s + BLK, M_C) - s if i == len(starts) - 1
                       else (starts[i + 1] - s))
                      for i, s in enumerate(starts)]
            blocks = [(s, min(e, M_C - s)) for s, e in blocks]
            flush_after = {blocks[min(k, len(blocks) - 1)][0]
                           for k in (7, 13, 19, 23, len(blocks) - 1)}

            # first input block before the (smaller) weight/bias loads, so
            # the PE pipeline fills as early as possible
            xth_first = xin.tile([IN_DIM, BLK], bf16, tag="xth")
            l0_0, lb_0 = blocks[0]
            nc.sync.dma_start(out=xth_first[:, :lb_0], in_=xth_d[:, :lb_0])

            wth_sb = singles.tile([IN_DIM, OUT_DIM], bf16)
            nc.sync.dma_start(out=wth_sb, in_=wth_d[:, :])
            bsc_sb = singles.tile([128, N_CHUNK], f32)
            nc.sync.dma_start(out=bsc_sb, in_=bsc_d[:, :])

            maxp_sb = accp.tile([128, N_CHUNK, G_C, TW], bf16)
            sump_sb = accp.tile([128, N_CHUNK, G_C, SW], bf16)

            warm_sb = singles.tile([128, 2], f32)
            nc.vector.memset(warm_sb, 0.0)
            nc.scalar.activation(
                out=warm_sb, in_=warm_sb,
                func=mybir.ActivationFunctionType.Relu, bias=0.0, scale=1.0,
            )

            def fold_tree(eng, pool, r4, out_ap, op, gb, tagp, levels=3):
                """16 -> 2 (or 4) halves-fold; inner step 1 keeps bf16 TT in
                2x_1P mode on DVE. The host folds the remaining tail."""
                t1 = pool.tile([128, N_CHUNK, gb, 8], bf16, tag=tagp + "1")
                eng.tensor_tensor(
                    out=t1, in0=r4[:, :, :, 0:8], in1=r4[:, :, :, 8:16], op=op)
                if levels == 2:
                    eng.tensor_tensor(
                        out=out_ap, in0=t1[:, :, :, 0:4], in1=t1[:, :, :, 4:8],
                        op=op)
                    return
                t2 = pool.tile([128, N_CHUNK, gb, 4], bf16, tag=tagp + "2")
                eng.tensor_tensor(
                    out=t2, in0=t1[:, :, :, 0:4], in1=t1[:, :, :, 4:8], op=op)
                eng.tensor_tensor(
                    out=out_ap, in0=t2[:, :, :, 0:2], in1=t2[:, :, :, 2:4], op=op)

            flush_from = 0
            for ib, (l0, lb) in enumerate(blocks):
                gb = lb // GS
                g0 = l0 // GS

                if ib == 0:
                    xth_sb = xth_first
                else:
                    xth_sb = xin.tile([IN_DIM, BLK], bf16, tag="xth")
                    nc.sync.dma_start(
                        out=xth_sb[:, :lb], in_=xth_d[:, l0 : l0 + lb])

                r_sb = rsb.tile([128, N_CHUNK, BLK], bf16, tag="r")
                n_wave = (lb + 511) // 512
                for c in range(N_CHUNK):
                    enc_ps = psum.tile([128, BLK], f32, tag="enc")
                    for w in range(n_wave):
                        w0 = w * 512
                        lw = min(512, lb - w0)
                        nc.tensor.matmul(
                            enc_ps[:, w0 : w0 + lw],
                            wth_sb[:, c * 128 : (c + 1) * 128],
                            xth_sb[:, w0 : w0 + lw],
                            start=True, stop=True,
                        )

                    span_idx = ib * N_CHUNK + c
                    if span_idx % DVE_EVAC_MOD == 5:
                        nc.vector.tensor_scalar(
                            out=r_sb[:, c, :lb],
                            in0=enc_ps[:, :lb],
                            scalar1=bsc_sb[:, c : c + 1],
                            scalar2=0.0,
                            op0=AL.add,
                            op1=AL.max,
                        )
                    else:
                        nc.scalar.activation(
                            out=r_sb[:, c, :lb],
                            in_=enc_ps[:, :lb],
                            func=mybir.ActivationFunctionType.Relu,
                            bias=bsc_sb[:, c : c + 1],
                            scale=1.0,
                        )

                r4 = r_sb[:, :, :lb].rearrange("p c (g s) -> p c g s", s=GS)
                fold_tree(
                    nc.vector, dtmp, r4,
                    maxp_sb[:, :, g0 : g0 + gb, :],
                    AL.max, gb, "m",
                )
                fold_tree(
                    nc.vector, dtmp, r4,
                    sump_sb[:, :, g0 : g0 + gb, :],
                    AL.add, gb, "s", levels=2,
                )

                if l0 in flush_after:
                    r0, r1 = flush_from, g0 + gb
                    flush_from = r1
                    for c in range(N_CHUNK):
                        nc.sync.dma_start(
                            out=omax_d[c * 128 : (c + 1) * 128, r0:r1, :],
                            in_=maxp_sb[:, c, r0:r1, :],
                        )
                        nc.sync.dma_start(
                            out=osum_d[c * 128 : (c + 1) * 128, r0:r1, :],
                            in_=sump_sb[:, c, r0:r1, :],
                        )

    nc.compile()
    return nc


def _get_nc(mode: str) -> bass.Bass:
    if mode not in _compiled:
        _compiled[mode] = _build(mode)
    return _compiled[mode]


def _host_prep(lane_encoding, W, b, mode: str):
    xT = np.ascontiguousarray(lane_encoding.T)
    wT = np.ascontiguousarray(W.T)
    bsc = np.ascontiguousarray(b.reshape(N_CHUNK, 128).T.astype(np.float32))

    in_maps = []
    bf = ml_dtypes.bfloat16
    xh = xT.astype(bf)
    wh = wT.astype(bf)
    for c in range(N_CORES):
        sl = slice(c * M_C, (c + 1) * M_C)
        in_maps.append({
            "xth": np.ascontiguousarray(xh[:, sl]),
            "wth": wh, "bsc": bsc,
        })
    return in_maps


def _run(lane_encoding, W, b, mode: str = MODE, trace: bool = False):
    nc = _get_nc(mode)
    in_maps = _host_prep(lane_encoding, W, b, mode)
    try:
        res = run_bass_kernel_spmd(
            nc, in_maps, core_ids=list(range(N_CORES)), trace=trace
        )
    except Exception:
        res = run_bass_kernel_spmd(
            nc, in_maps, core_ids=list(range(N_CORES)), trace=trace
        )
    out = np.empty((N_OBS, 2 * OUT_DIM), dtype=np.float32)
    for c in range(N_CORES):
        gsl = slice(c * G_C, (c + 1) * G_C)
        om = res.results[c]["omax"].astype(np.float32)  # [512, G_C, TW]
        os_ = res.results[c]["osum"].astype(np.float32)
        out[gsl, :OUT_DIM] = om.max(axis=2).T
        out[gsl, OUT_DIM:] = os_.sum(axis=2).T / GS
    return out, res


def kernel(obs_encoding, lane_encoding, same_obs_mask, W, b):
    out, _ = _run(
        np.asarray(lane_encoding, dtype=np.float32),
        np.asarray(W, dtype=np.float32),
        np.asarray(b, dtype=np.float32),
        MODE,
    )
    return out


# revision 30
# speedup vs baseline: 1.1546x; 1.1546x over previous
"""Streaming-block GNN pool kernel (best config): bf16 single-pass matmul,
ACT relu+bias evac to bf16, all-DVE half-fold tensor_tensor trees (2x_1P
bf16 = 4 reads/cycle) reducing each 16-group to 2 partials; the host folds
the last pair and converts sum -> mean. GPSIMD stays idle: its SBUF port is
shared with the DVE and concurrent use inflates both by 30-70%."""
import sys

sys.path.insert(0, "/opt/trn_rl_repo")

import numpy as np
import ml_dtypes

import concourse.bass as bass
import concourse.bacc as bacc
import concourse.tile as tile
from concourse import mybir
from concourse.bass_utils import run_bass_kernel_spmd

N_CORES = 8
IN_DIM = 128
OUT_DIM = 512
N_OBS = 25000
M_LANES = 400000
GS = 16
M_C = M_LANES // N_CORES
G_C = N_OBS // N_CORES
N_CHUNK = OUT_DIM // 128
BLK = 2048
TW = 2   # tail width left for the host fold
SW = 2   # sum tail width

MODE = "treev10"
DVE_EVAC_MOD = 0  # disabled: ACT and DVE are balanced at SW=2

_compiled = {}


def _build(mode: str) -> bass.Bass:
    nc = bacc.Bacc(None, target_bir_lowering=False)
    f32 = mybir.dt.float32
    bf16 = mybir.dt.bfloat16
    AL = mybir.AluOpType

    xth_d = nc.dram_tensor("xth", [IN_DIM, M_C], bf16, kind="ExternalInput")
    wth_d = nc.dram_tensor("wth", [IN_DIM, OUT_DIM], bf16, kind="ExternalInput")
    bsc_d = nc.dram_tensor("bsc", [128, N_CHUNK], f32, kind="ExternalInput")
    omax_d = nc.dram_tensor("omax", [OUT_DIM, G_C, TW], bf16, kind="ExternalOutput")
    osum_d = nc.dram_tensor("osum", [OUT_DIM, G_C, SW], bf16, kind="ExternalOutput")

    with nc.allow_low_precision("bf16 pooled outputs; host upconverts"), \
            tile.TileContext(nc) as tc:
        with (
            tc.tile_pool(name="singles", bufs=1) as singles,
            tc.tile_pool(name="xin", bufs=3) as xin,
            tc.tile_pool(name="rsb", bufs=2) as rsb,
            tc.tile_pool(name="acc", bufs=1) as accp,
            tc.tile_pool(name="dtmp", bufs=2) as dtmp,
            tc.tile_pool(name="psum", bufs=2, space="PSUM") as psum,
        ):
            starts = [0, 512]
            while starts[-1] + BLK < M_C:
                starts.append(starts[-1] + BLK)
            blocks = [(s, min(s + BLK, M_C) - s if i == len(starts) - 1
                       else (starts[i + 1] - s))
                      for i, s in enumerate(starts)]
            blocks = [(s, min(e, M_C - s)) for s, e in blocks]
            flush_after = {blocks[min(k, len(blocks) - 1)][0]
                           for k in (7, 13, 19, 23, len(blocks) - 1)}

            # first input block before the (smaller) weight/bias loads, so
            # the PE pipeline fills as early as possible
            xth_first = xin.tile([IN_DIM, BLK], bf16, tag="xth")
            l0_0, lb_0 = blocks[0]
            nc.sync.dma_start(out=xth_first[:, :lb_0], in_=xth_d[:, :lb_0])

            wth_sb = singles.tile([IN_DIM, OUT_DIM], bf16)
            nc.sync.dma_start(out=wth_sb, in_=wth_d[:, :])
            bsc_sb = singles.tile([128, N_CHUNK], f32)
            nc.sync.dma_start(out=bsc_sb, in_=bsc_d[:, :])

            maxp_sb = accp.tile([128, N_CHUNK, G_C, TW], bf16)
            sump_sb = accp.tile([128, N_CHUNK, G_C, SW], bf16)

            warm_sb = singles.tile([128, 2], f32)
            nc.vector.memset(warm_sb, 0.0)
            nc.scalar.activation(
                out=warm_sb, in_=warm_sb,
                func=mybir.ActivationFunctionType.Relu, bias=0.0, scale=1.0,
            )

            def fold_tree(eng, pool, r4, out_ap, op, gb, tagp, levels=3):
                """16 -> 2 (or 4) halves-fold; inner step 1 keeps bf16 TT in
                2x_1P mode on DVE. The host folds the remaining tail."""
                t1 = pool.tile([128, N_CHUNK, gb, 8], bf16, tag=tagp + "1")
                eng.tensor_tensor(
                    out=t1, in0=r4[:, :, :, 0:8], in1=r4[:, :, :, 8:16], op=op)
                if levels == 2:
                    eng.tensor_tensor(
                        out=out_ap, in0=t1[:, :, :, 0:4], in1=t1[:, :, :, 4:8],
                        op=op)
                    return
                t2 = pool.tile([128, N_CHUNK, gb, 4], bf16, tag=tagp + "2")
                eng.tensor_tensor(
                    out=t2, in0=t1[:, :, :, 0:4], in1=t1[:, :, :, 4:8], op=op)
                eng.tensor_tensor(
                    out=out_ap, in0=t2[:, :, :, 0:2], in1=t2[:, :, :, 2:4], op=op)

            flush_from = 0
            for ib, (l0, lb) in enumerate(blocks):
                gb = lb // GS
                g0 = l0 // GS

                if ib == 0:
                    xth_sb = xth_first
                else:
                    xth_sb = xin.tile([IN_DIM, BLK], bf16, tag="xth")
                    nc.sync.dma_start(
                        out=xth_sb[:, :lb], in_=xth_d[:, l0 : l0 + lb])

                r_sb = rsb.tile([128, N_CHUNK, BLK], bf16, tag="r")
                n_wave = (lb + 511) // 512
                for c in range(N_CHUNK):
                    enc_ps = psum.tile([128, BLK], f32, tag="enc")
                    for w in range(n_wave):
                        w0 = w * 512
                        lw = min(512, lb - w0)
                        nc.tensor.matmul(
                            enc_ps[:, w0 : w0 + lw],
                            wth_sb[:, c * 128 : (c + 1) * 128],
                            xth_sb[:, w0 : w0 + lw],
                            start=True, stop=True,
                        )

                    nc.scalar.activation(
                        out=r_sb[:, c, :lb],
                        in_=enc_ps[:, :lb],
                        func=mybir.ActivationFunctionType.Relu,
                        bias=bsc_sb[:, c : c + 1],
                        scale=1.0,
                    )

                r4 = r_sb[:, :, :lb].rearrange("p c (g s) -> p c g s", s=GS)
                fold_tree(
                    nc.vector, dtmp, r4,
                    maxp_sb[:, :, g0 : g0 + gb, :],
                    AL.max, gb, "m",
                )
                fold_tree(
                    nc.vector, dtmp, r4,
                    sump_sb[:, :, g0 : g0 + gb, :],
                    AL.add, gb, "s",
                )

                if l0 in flush_after:
                    r0, r1 = flush_from, g0 + gb
                    flush_from = r1
                    for c in range(N_CHUNK):
                        nc.sync.dma_start(
                            out=omax_d[c * 128 : (c + 1) * 128, r0:r1, :],
                            in_=maxp_sb[:, c, r0:r1, :],
                        )
                        nc.sync.dma_start(
                            out=osum_d[c * 128 : (c + 1) * 128, r0:r1, :],
                            in_=sump_sb[:, c, r0:r1, :],
                        )

    nc.compile()
    return nc


def _get_nc(mode: str) -> bass.Bass:
    if mode not in _compiled:
        _compiled[mode] = _build(mode)
    return _compiled[mode]


def _host_prep(lane_encoding, W, b, mode: str):
    xT = np.ascontiguousarray(lane_encoding.T)
    wT = np.ascontiguousarray(W.T)
    bsc = np.ascontiguousarray(b.reshape(N_CHUNK, 128).T.astype(np.float32))

    in_maps = []
    bf = ml_dtypes.bfloat16
    xh = xT.astype(bf)
    wh = wT.astype(bf)
    for c in range(N_CORES):
        sl = slice(c * M_C, (c + 1) * M_C)
        in_maps.append({
            "xth": np.ascontiguousarray(xh[:, sl]),
            "wth": wh, "bsc": bsc,
        })
    return in_maps


def _run(lane_encoding, W, b, mode: str = MODE, trace: bool = False):
    nc = _get_nc(mode)
    in_maps = _host_prep(lane_encoding, W, b, mode)
    try:
        res = run_bass_kernel_spmd(
            nc, in_maps, core_ids=list(range(N_CORES)), trace=trace
        )
    except Exception:
        res = run_bass_kernel_spmd(
            nc, in_maps, core_ids=list(range(N_CORES)), trace=trace
        )
    out = np.empty((N_OBS, 2 * OUT_DIM), dtype=np.float32)
    for c in range(N_CORES):
        gsl = slice(c * G_C, (c + 1) * G_C)
        om = res.results[c]["omax"].astype(np.float32)  # [512, G_C, TW]
        os_ = res.results[c]["osum"].astype(np.float32)
        out[gsl, :OUT_DIM] = om.max(axis=2).T
        out[gsl, OUT_DIM:] = os_.sum(axis=2).T / GS
    return out, res


def kernel(obs_encoding, lane_encoding, same_obs_mask, W, b):
    out, _ = _run(
        np.asarray(lane_encoding, dtype=np.float32),
        np.asarray(W, dtype=np.float32),
        np.asarray(b, dtype=np.float32),
        MODE,
    )
    return out
